# revision 8
# baseline (speedup 1.0000x reference)
"""BiLSTM kernel for Trainium2 (Bass/Tile), 8-core data-parallel.

Reference computation (per batch row):
  xp_f = X @ w_ih_f^T + b_ih_f + b_hh_f          [S, 20] gate preacts
  forward LSTM scan over S=100 steps -> h_f       [5]
  backward dir: single cell at s=S-1 -> h_b       [5]
  out = [h_f, h_b] @ fc_w^T + fc_b                [1000]

Sharding: batch 1200 -> 150 per core, weights replicated.

Kernel layout choices:
  - gates kept "transposed": [20 gates, batch] on partitions; X tiles are
    PE-transposed on chip so D sits on partitions for the big matmul.
  - gate order permuted to [i, f, o, g] on host so sigmoid covers rows 0:15
    in one ACT op and tanh covers 15:20 (same HW activation table).
  - streaming matmuls run as float32r (full-rate fp32 mode, N>=256).
"""

import sys

sys.path.insert(0, "/opt/trn_rl_repo")

import math
from contextlib import ExitStack

import numpy as np

import concourse.bass as bass
import concourse.bacc as bacc
import concourse.mybir as mybir
import concourse.tile as tile
from concourse.bass_utils import run_bass_kernel_spmd
from concourse.masks import make_identity

F32 = mybir.dt.float32
F32R = mybir.dt.float32r
AF = mybir.ActivationFunctionType

N_CORES = 8
B = 150          # batch per core
S = 100          # timesteps
D = 1000         # input dim
H = 5            # hidden
G = 4 * H        # gates
GW = 101         # gate rows spread to 32-aligned bases: i@0, f@32, o@64, g@96
GOFF = (0, 32, 64, 96)   # i, f, o, g row offsets
DCH = 125        # d-chunk width (8 uniform chunks of 125)
NCH = D // DCH
GSTEP = 6        # timesteps per streaming group
HSTEP = 3        # timesteps per PSUM gate tile (3*150=450 <= 512)

# gate permutation: torch order [i, f, g, o] -> ours [i, f, o, g]
GATE_PERM = list(range(0, 10)) + list(range(15, 20)) + list(range(10, 15))


def build_nc():
    nc = bacc.Bacc(None, target_bir_lowering=False)

    x = nc.declare_dram_parameter("x", [B, S, D], F32, isOutput=False)
    wTf = nc.declare_dram_parameter("wTf", [D, GW], F32, isOutput=False)
    wTb = nc.declare_dram_parameter("wTb", [D, GW], F32, isOutput=False)
    whhTf = nc.declare_dram_parameter("whhTf", [H, GW], F32, isOutput=False)
    whhTb = nc.declare_dram_parameter("whhTb", [H, GW], F32, isOutput=False)
    # biases split so every ACT op reads a base-partition-0 tile
    bsig_f = nc.declare_dram_parameter("bsig_f", [96, 1], F32, isOutput=False)
    btan_f = nc.declare_dram_parameter("btan_f", [101, 1], F32, isOutput=False)
    bsig_b = nc.declare_dram_parameter("bsig_b", [96, 1], F32, isOutput=False)
    btan_b = nc.declare_dram_parameter("btan_b", [101, 1], F32, isOutput=False)
    fcwf = nc.declare_dram_parameter("fcwf", [H, D], F32, isOutput=False)
    fcwb = nc.declare_dram_parameter("fcwb", [H, D], F32, isOutput=False)
    fcb = nc.declare_dram_parameter("fcb", [1, D], F32, isOutput=False)
    h0fT = nc.declare_dram_parameter("h0fT", [H, B], F32, isOutput=False)
    c0fT = nc.declare_dram_parameter("c0fT", [H, B], F32, isOutput=False)
    h0bT = nc.declare_dram_parameter("h0bT", [H, B], F32, isOutput=False)
    c0bT = nc.declare_dram_parameter("c0bT", [H, B], F32, isOutput=False)
    out = nc.declare_dram_parameter("out", [B, D], F32, isOutput=True)

    with tile.TileContext(nc) as tc:
        with ExitStack() as ctx:
            _emit(nc, tc, ctx, locals())
    nc.finalize()
    return nc


def _emit(nc, tc, ctx, t):
    x, out = t["x"], t["out"]

    consts = ctx.enter_context(tc.tile_pool(name="consts", bufs=1))
    state = ctx.enter_context(tc.tile_pool(name="state", bufs=2))
    upool = ctx.enter_context(tc.tile_pool(name="u", bufs=3))
    staging = ctx.enter_context(tc.tile_pool(name="staging", bufs=12))
    xt_pool = ctx.enter_context(tc.tile_pool(name="xt", bufs=2))
    osb_pool = ctx.enter_context(tc.tile_pool(name="osb", bufs=2))
    tpsum = ctx.enter_context(tc.tile_pool(name="tpsum", bufs=2, space="PSUM"))
    xppsum = ctx.enter_context(tc.tile_pool(name="xp", bufs=4, space="PSUM"))
    fcpsum = ctx.enter_context(tc.tile_pool(name="fc", bufs=2, space="PSUM"))

    ident = consts.tile([128, 128], F32)
    make_identity(nc, ident)

    wf_ld = consts.tile([DCH, NCH, GW], F32)
    nc.sync.dma_start(out=wf_ld, in_=t["wTf"].rearrange("(c p) g -> p c g", p=DCH))
    wb_ld = consts.tile([DCH, NCH, GW], F32)
    nc.sync.dma_start(out=wb_ld, in_=t["wTb"].rearrange("(c p) g -> p c g", p=DCH))
    wf_sb = consts.tile([DCH, NCH, GW], F32R)
    nc.vector.tensor_copy(out=wf_sb, in_=wf_ld)
    wb_sb = consts.tile([DCH, NCH, GW], F32R)
    nc.vector.tensor_copy(out=wb_sb, in_=wb_ld)

    whhf_sb = consts.tile([H, GW], F32)
    nc.sync.dma_start(out=whhf_sb, in_=t["whhTf"][:])
    whhb_sb = consts.tile([H, GW], F32)
    nc.sync.dma_start(out=whhb_sb, in_=t["whhTb"][:])

    bias_sb = {}
    for name, rows in (("bsig_f", 96), ("btan_f", 101), ("bsig_b", 96), ("btan_b", 101)):
        bias_sb[name] = consts.tile([rows, 1], F32, name=f"bias_{name}")
        nc.sync.dma_start(out=bias_sb[name], in_=t[name][:])

    fcwf_ld = consts.tile([H, D], F32)
    nc.sync.dma_start(out=fcwf_ld, in_=t["fcwf"][:])
    fcwb_ld = consts.tile([H, D], F32)
    nc.sync.dma_start(out=fcwb_ld, in_=t["fcwb"][:])
    fcwf_sb = consts.tile([H, D], F32R)
    nc.vector.tensor_copy(out=fcwf_sb, in_=fcwf_ld)
    fcwb_sb = consts.tile([H, D], F32R)
    nc.vector.tensor_copy(out=fcwb_sb, in_=fcwb_ld)

    fcb_sb = consts.tile([128, D], F32)
    fcb_bcast = bass.AP(
        tensor=t["fcb"][:].tensor,
        offset=t["fcb"][:].offset,
        ap=[[0, 128]] + list(t["fcb"][:].ap[1:]),
    )
    nc.gpsimd.dma_start(out=fcb_sb, in_=fcb_bcast)

    hT = state.tile([H, B], F32, tag="h")
    nc.sync.dma_start(out=hT, in_=t["h0fT"][:])
    cT = state.tile([37, B], F32, tag="c")
    nc.sync.dma_start(out=cT[32:37, :], in_=t["c0fT"][:])
    h0b_sb = consts.tile([H, B], F32)
    nc.sync.dma_start(out=h0b_sb, in_=t["h0bT"][:])
    c0b_sb = consts.tile([37, B], F32)
    nc.sync.dma_start(out=c0b_sb[32:37, :], in_=t["c0bT"][:])

    def cell(xp_slice, whh_sb, h_prev, bsig, btan, c_prev):
        """One LSTM cell on a [G, width] psum gate slice; returns (h, c)."""
        width = xp_slice.shape[-1]
        # recurrent term accumulates on top of the streamed input projection
        nc.tensor.matmul(xp_slice, whh_sb, h_prev, start=False, stop=True)
        us = upool.tile([96, B], F32, tag="u")
        nc.scalar.activation(out=us[:, :width], in_=xp_slice[0:96], func=AF.Sigmoid,
                             bias=bsig)
        ug = upool.tile([H, B], F32, tag="ug")
        nc.scalar.activation(out=ug[:, :width], in_=xp_slice[96:101], func=AF.Tanh,
                             bias=btan[96:101])
        tmp = upool.tile([37, B], F32, tag="tmp")
        nc.vector.tensor_mul(tmp[32:37, :width], us[0:5, :width], ug[:, :width])
        m1 = upool.tile([37, B], F32, tag="m1")
        nc.vector.tensor_mul(m1[32:37, :width], us[32:37, :width],
                             c_prev[32:37, :width])
        c_new = state.tile([37, B], F32, tag="c")
        nc.vector.tensor_add(c_new[32:37, :width], m1[32:37, :width],
                             tmp[32:37, :width])
        tch = upool.tile([69, B], F32, tag="tc")
        nc.scalar.activation(out=tch[64:69, :width], in_=c_new[32:37, :width],
                             func=AF.Tanh)
        h_new = state.tile([H, B], F32, tag="h")
        nc.vector.tensor_mul(h_new[:, :width], us[64:69, :width], tch[64:69, :width])
        return h_new, c_new

    n_groups = math.ceil(S / GSTEP)
    xt_last = None
    for g in range(n_groups):
        s0 = g * GSTEP
        gsteps = min(GSTEP, S - s0)
        tok = B * gsteps
        nblk = math.ceil(tok / 128)

        # --- stage X rows (token-major: tau = s_local*B + b) ---
        stg = []
        for j in range(nblk):
            t0, t1 = 128 * j, min(128 * (j + 1), tok)
            st = staging.tile([128, D], F32, tag="stg")
            stg.append((st, t1 - t0))
            tau = t0
            while tau < t1:
                sl, b0 = divmod(tau, B)
                run = min(B - b0, t1 - tau)
                nc.sync.dma_start(out=st[tau - t0:tau - t0 + run, :],
                                  in_=x[b0:b0 + run, s0 + sl, :])
                tau += run

        # --- transpose into [DCH, NCH, tok] ---
        xt = xt_pool.tile([DCH, NCH, B * GSTEP], F32R, tag="xt")
        if g == n_groups - 1:
            xt_last = xt
        for k in range(NCH):
            for half in range(math.ceil(nblk / 4)):
                jlo = 4 * half
                jhi = min(jlo + 4, nblk)
                c0 = 128 * jlo
                width = min(128 * jhi, tok) - c0
                pt = tpsum.tile([DCH, 512], F32, tag="tp")
                for j in range(jlo, jhi):
                    st, w = stg[j]
                    off = 128 * (j - jlo)
                    nc.tensor.transpose(pt[:, off:off + w],
                                        st[:w, k * DCH:(k + 1) * DCH],
                                        ident[:w, :w])
                nc.vector.tensor_copy(out=xt[:, k, c0:c0 + width],
                                      in_=pt[:, :width])

        # --- gate preacts + recurrence, per 3-step half ---
        for h0 in range(0, gsteps, HSTEP):
            steps_h = min(HSTEP, gsteps - h0)
            c0 = h0 * B
            width = steps_h * B
            xp = xppsum.tile([GW, HSTEP * B], F32, tag="xp")
            for k in range(NCH):
                nc.tensor.matmul(xp[:, :width],
                                 wf_sb[:, k, :],
                                 xt[:, k, c0:c0 + width],
                                 start=(k == 0), stop=(k == NCH - 1))
            for si in range(steps_h):
                hT, cT = cell(xp[:, si * B:(si + 1) * B], whhf_sb, hT,
                              bias_sb["bsig_f"], bias_sb["btan_f"], cT)

    # --- backward direction: one cell on the last timestep's tokens ---
    lg_tok0 = (S - 1 - (n_groups - 1) * GSTEP) * B
    xpb = xppsum.tile([GW, HSTEP * B], F32, tag="xp")
    for k in range(NCH):
        nc.tensor.matmul(xpb[:, :B],
                         wb_sb[:, k, :],
                         xt_last[:, k, lg_tok0:lg_tok0 + B],
                         start=(k == 0), stop=(k == NCH - 1))
    hbT, _ = cell(xpb[:, :B], whhb_sb, h0b_sb, bias_sb["bsig_b"],
                  bias_sb["btan_b"], c0b_sb)

    # --- FC: out[b, n] = sum_k hf[k,b] fcwf[k,n] + sum_k hb[k,b] fcwb[k,n] + fcb ---
    hf_r = consts.tile([H, B], F32R)
    nc.vector.tensor_copy(out=hf_r, in_=hT)
    hb_r = consts.tile([H, B], F32R)
    nc.vector.tensor_copy(out=hb_r, in_=hbT)
    NW = 500
    for b0, bw in ((0, 128), (128, B - 128)):
        osb = osb_pool.tile([128, D], F32, tag="osb")
        for n0 in range(0, D, NW):
            ps = fcpsum.tile([128, NW], F32, tag="fc")
            nc.tensor.matmul(ps[:bw, :], hf_r[:, b0:b0 + bw],
                             fcwf_sb[:, n0:n0 + NW].bitcast(F32R),
                             start=True, stop=False)
            nc.tensor.matmul(ps[:bw, :], hb_r[:, b0:b0 + bw],
                             fcwb_sb[:, n0:n0 + NW].bitcast(F32R),
                             start=False, stop=True)
            nc.vector.tensor_add(osb[:bw, n0:n0 + NW], ps[:bw, :],
                                 fcb_sb[:bw, n0:n0 + NW])
        nc.sync.dma_start(out=out[b0:b0 + bw, :], in_=osb[:bw, :])


def prep_in_maps(X, h0, c0, w_ih_f, w_hh_f, b_ih_f, b_hh_f,
                 w_ih_b, w_hh_b, b_ih_b, b_hh_b, fc_w, fc_b):
    """Host-side prep: slice batch per core, permute gates, transpose weights."""
    def f32(a):
        return np.ascontiguousarray(np.asarray(a, dtype=np.float32))

    def spread_w(w):
        # w [20, cols] torch gate order [i, f, g, o] -> [cols, GW] with
        # i@0:5, f@32:37, o@64:69, g@96:101
        w = np.asarray(w)
        out_ = np.zeros((w.shape[1], GW), dtype=np.float32)
        for gi, (src_lo, dst) in enumerate(((0, 0), (5, 32), (15, 64), (10, 96))):
            out_[:, dst:dst + H] = w[src_lo:src_lo + H].T
        return f32(out_)

    def spread_bias(b):
        b = np.asarray(b)
        sig = np.zeros((96, 1), dtype=np.float32)
        sig[0:5, 0] = b[0:5]      # i
        sig[32:37, 0] = b[5:10]   # f
        sig[64:69, 0] = b[15:20]  # o
        tan = np.zeros((101, 1), dtype=np.float32)
        tan[96:101, 0] = b[10:15]  # g
        return f32(sig), f32(tan)

    shared = {
        "wTf": spread_w(w_ih_f),
        "wTb": spread_w(w_ih_b),
        "whhTf": spread_w(w_hh_f),
        "whhTb": spread_w(w_hh_b),
        "fcwf": f32(np.asarray(fc_w).T[0:H]),
        "fcwb": f32(np.asarray(fc_w).T[H:2 * H]),
        "fcb": f32(np.asarray(fc_b).reshape(1, D)),
    }
    shared["bsig_f"], shared["btan_f"] = spread_bias(np.asarray(b_ih_f) + np.asarray(b_hh_f))
    shared["bsig_b"], shared["btan_b"] = spread_bias(np.asarray(b_ih_b) + np.asarray(b_hh_b))

    X = np.asarray(X, dtype=np.float32)
    h0 = np.asarray(h0, dtype=np.float32)
    c0 = np.asarray(c0, dtype=np.float32)
    in_maps = []
    for c in range(N_CORES):
        sl = slice(c * B, (c + 1) * B)
        m = dict(shared)
        m["x"] = np.ascontiguousarray(X[sl])
        m["h0fT"] = f32(h0[0, sl].T)
        m["c0fT"] = f32(c0[0, sl].T)
        m["h0bT"] = f32(h0[1, sl].T)
        m["c0bT"] = f32(c0[1, sl].T)
        in_maps.append(m)
    return in_maps


_NC_CACHE = None


def get_nc():
    global _NC_CACHE
    if _NC_CACHE is None:
        _NC_CACHE = build_nc()
    return _NC_CACHE


def run(in_maps, **kw):
    nc = get_nc()
    return run_bass_kernel_spmd(nc, in_maps, list(range(N_CORES)), **kw)


def kernel(**inputs):
    in_maps = prep_in_maps(**inputs)
    res = run(in_maps)
    return np.concatenate([res.results[c]["out"] for c in range(N_CORES)], axis=0)
